# revision 1
# baseline (speedup 1.0000x reference)
"""Cross-multi-head-attention (causal) Trainium2 Bass kernel, v2.

Problem: B=4, T=2048, C=2048, 16 heads x head_dim 128.
  kv = enc_x @ W_kv + b_kv ; q = dec_x @ W_q + b_q
  out = softmax_causal(q k^T / sqrt(hd)) v  -> concat heads -> @ W_o + b_o

Sharding over 8 cores: core c -> (batch b = c//2, head-group hg = c%2 of 8
heads). Host sums the two partials per batch and adds b_o.

v2 design vs v1:
- bf16 operands throughout (PE rate identical to f32r, half the DMA/SBUF,
  no narrow-matmul f32r penalty). PSUM accumulation stays f32.
- K^T, O^T, xdT and wq SBUF-resident; V streams through a DRAM scratch with
  one consolidated (rearranged-AP) DMA per head.
- Q projection matmuls for head h+1 are woven between the attention score
  strips of head h, so the PE always has queue-ready work while the
  activation engine catches up on exp.
- Softmax denominators: DVE/Pool adds over exp tiles, then
  gpsimd.partition_all_reduce + partition_broadcast; no PE matmuls for
  reductions or broadcasts.
- DMAs are consolidated (each dma_start costs ~625ns on the serialized
  HWDGE ring) and the K-phase weight/x loads are interleaved so the first
  matmul starts ~2us in.
"""
import sys

sys.path.insert(0, "/opt/trn_rl_repo")

import numpy as np

DIM = 2048
N_HEAD = 16
HEAD = DIM // N_HEAD  # 128
B = 4
T = 2048
HPC = 8               # heads per core
KC = HPC * HEAD       # 1024 projected cols per core
SCALE = 1.0 / np.sqrt(float(HEAD))
N_CORES = 8


def _build(t=T, add_bias_kq=False):
    from contextlib import ExitStack

    import concourse.mybir as mybir
    from concourse import bacc
    from concourse import bass_isa
    from concourse.tile import TileContext

    F32 = mybir.dt.float32
    BF16 = mybir.dt.bfloat16
    AF = mybir.ActivationFunctionType
    RADD = bass_isa.ReduceOp.add

    n_tb = t // 512      # 512-col T blocks
    n_tc = t // 128      # 128-row T chunks
    n_g = t // 512       # q groups in attention

    nc = bacc.Bacc("TRN2", target_bir_lowering=False, debug=False, num_devices=1)
    xeT = nc.dram_tensor("xeT", [DIM, t], BF16, kind="ExternalInput").ap()
    xdT = nc.dram_tensor("xdT", [DIM, t], BF16, kind="ExternalInput").ap()
    wk = nc.dram_tensor("wk", [DIM, KC], BF16, kind="ExternalInput").ap()
    wv = nc.dram_tensor("wv", [DIM, KC], BF16, kind="ExternalInput").ap()
    wq = nc.dram_tensor("wq", [DIM, KC], BF16, kind="ExternalInput").ap()
    wo = nc.dram_tensor("wo", [KC, DIM], BF16, kind="ExternalInput").ap()
    tri = nc.dram_tensor("tri", [128, 128], BF16, kind="ExternalInput").ap()
    if add_bias_kq:
        bk = nc.dram_tensor("bk", [KC, 1], F32, kind="ExternalInput").ap()
        bq = nc.dram_tensor("bq", [KC, 1], F32, kind="ExternalInput").ap()
        bvb = nc.dram_tensor("bvb", [128, KC], F32, kind="ExternalInput").ap()
    out = nc.dram_tensor("out", [t, DIM], F32, kind="ExternalOutput").ap()

    v_s = nc.dram_tensor("v_s", [t, KC], BF16, kind="Internal").ap()
    # [128p, 16i, 128c] view: element (p,i,c) = v_s[i*128+p, c]
    v_s_r = v_s.rearrange("(i p) c -> p i c", p=128)
    # [128p, 4j, t] views of x chunks: element (p,j,u) = x[(s*4+j)*128+p, u]
    xeT_r = xeT.rearrange("(s j p) u -> s p j u", j=4, p=128)

    with TileContext(nc) as tc, ExitStack() as top:
        glob = top.enter_context(tc.tile_pool(name="glob", bufs=1))
        tri_sb = glob.tile([128, 128], BF16, tag="tri", name="tri_sb")
        nc.sync.dma_start(out=tri_sb, in_=tri)
        bk_b = bq_b = bvb_sb = None
        if add_bias_kq:
            bk_sb = glob.tile([128, HPC], F32, tag="bk", name="bk_sb")
            bq_sb = glob.tile([128, HPC], F32, tag="bq", name="bq_sb")
            bvb_sb = glob.tile([128, KC], F32, tag="bvb", name="bvb_sb")
            for h in range(HPC):
                nc.sync.dma_start(out=bk_sb[:, h:h + 1],
                                  in_=bk[h * 128:(h + 1) * 128, :])
                nc.sync.dma_start(out=bq_sb[:, h:h + 1],
                                  in_=bq[h * 128:(h + 1) * 128, :])
            nc.sync.dma_start(out=bvb_sb, in_=bvb)
            bk_b = [bk_sb[:, h:h + 1] for h in range(HPC)]
            bq_b = [bq_sb[:, h:h + 1] for h in range(HPC)]

        # ---- persistent SBUF residents ----
        ktp = top.enter_context(tc.tile_pool(name="ktp", bufs=1))
        otp = top.enter_context(tc.tile_pool(name="otp", bufs=1))
        kt_h = [ktp.tile([128, t], BF16, tag=f"kt{h}", name=f"kt{h}")
                for h in range(HPC)]
        ot_h = [otp.tile([128, t], BF16, tag=f"ot{h}", name=f"ot{h}")
                for h in range(HPC)]

        # Q-side residents (xdT chunks + wq), freed before wo loads
        qside = ExitStack()
        xdp = qside.enter_context(tc.tile_pool(name="xdp", bufs=1, side="right"))
        wqp = qside.enter_context(tc.tile_pool(name="wqp", bufs=1, side="right"))
        xd_t = [xdp.tile([128, t], BF16, tag=f"xd{c}", name=f"xd{c}")
                for c in range(16)]
        wq_t = [wqp.tile([128, KC], BF16, tag=f"wq{c}", name=f"wq{c}")
                for c in range(16)]

        # ---- K/V projection (streamed xeT, bf16 weights) ----
        proj = ExitStack()
        xp = proj.enter_context(tc.tile_pool(name="px", bufs=2, side="right"))
        wp = proj.enter_context(tc.tile_pool(name="pw", bufs=16, side="right"))
        vo = proj.enter_context(tc.tile_pool(name="pvo", bufs=3, side="right"))
        pp = proj.enter_context(tc.tile_pool(name="pps", bufs=8, space="PSUM"))

        def x_super(tb, s, pfx):
            """One DMA bringing xeT chunks 4s..4s+3, cols [tb*512,(tb+1)*512)
            into a [128, 4, 512] tile viewed as [128, 2048]."""
            x1 = xp.tile([128, 4 * 512], BF16, tag="x", name=f"{pfx}x{tb}_{s}")
            nc.sync.dma_start(
                out=x1.rearrange("p (j u) -> p j u", j=4),
                in_=xeT_r[s][:, :, tb * 512:(tb + 1) * 512])
            return x1

        def k_proj():
            wts = [None] * 16
            xts = [None] * 4
            # interleave x/weight loads so matmuls can start ~2us in
            for s in range(4):
                xts[s] = x_super(0, s, "k")
                for c in range(4 * s, 4 * s + 4):
                    wt = wp.tile([128, KC], BF16, tag="w", name=f"kw_{c}")
                    nc.sync.dma_start(out=wt, in_=wk[c * 128:(c + 1) * 128, :])
                    wts[c] = wt
            for tb in range(n_tb):
                if tb > 0:
                    xts = [x_super(tb, s, "k") for s in range(4)]
                ps = [pp.tile([128, 512], F32, tag="p", name=f"kp{tb}_{h}")
                      for h in range(HPC)]
                c_order = list(range(16))
                for ci, c in enumerate(c_order):
                    xr = xts[c // 4][:, (c % 4) * 512:(c % 4 + 1) * 512]
                    for h in range(HPC):
                        nc.tensor.matmul(
                            ps[h], wts[c][:, h * 128:(h + 1) * 128], xr,
                            start=(ci == 0), stop=(ci == 15))
                for h in range(HPC):
                    dst = kt_h[h][:, tb * 512:(tb + 1) * 512]
                    if bk_b is not None:
                        nc.scalar.activation(dst, ps[h], AF.Identity,
                                             bias=bk_b[h])
                    elif h % 2 == 0:
                        nc.scalar.activation(dst, ps[h], AF.Identity)
                    else:
                        nc.vector.tensor_copy(dst, ps[h])

        def v_proj():
            xts0 = [x_super(0, s, "v") for s in range(4)]
            wts = []
            for c in range(16):
                wt = wp.tile([128, KC], BF16, tag="w", name=f"vw_{c}")
                nc.sync.dma_start(out=wt, in_=wv[c * 128:(c + 1) * 128, :])
                wts.append(wt)
            # prefetch Q-side residents behind the wv loads
            for c in range(16):
                nc.sync.dma_start(out=wq_t[c],
                                  in_=wq[c * 128:(c + 1) * 128, :])
                nc.sync.dma_start(out=xd_t[c],
                                  in_=xdT[c * 128:(c + 1) * 128, :])
            for tb in range(n_tb):
                xts = xts0 if tb == 0 else [x_super(tb, s, "v") for s in range(4)]
                ps = [pp.tile([128, 512], F32, tag="p", name=f"vp{tb}_{j}")
                      for j in range(8)]
                for c in range(16):
                    xr = xts[c // 4][:, (c % 4) * 512:(c % 4 + 1) * 512]
                    for ts in range(4):
                        for vg in range(2):
                            nc.tensor.matmul(
                                ps[ts * 2 + vg],
                                xr[:, ts * 128:(ts + 1) * 128],
                                wts[c][:, vg * 512:(vg + 1) * 512],
                                start=(c == 0), stop=(c == 15))
                for ts in range(4):
                    st = vo.tile([128, KC], BF16, tag="vo", name=f"vo{tb}_{ts}")
                    for vg in range(2):
                        seg = st[:, vg * 512:(vg + 1) * 512]
                        if bvb_sb is not None:
                            nc.vector.tensor_add(
                                seg, ps[ts * 2 + vg],
                                bvb_sb[:, vg * 512:(vg + 1) * 512])
                        elif (ts * 2 + vg) % 2 == 0:
                            nc.scalar.activation(seg, ps[ts * 2 + vg],
                                                 AF.Identity)
                        else:
                            nc.vector.tensor_copy(seg, ps[ts * 2 + vg])
                    nc.sync.dma_start(
                        out=v_s[tb * 512 + ts * 128:tb * 512 + (ts + 1) * 128, :],
                        in_=st)

        with tc.spectator_scope("p_k"):
            k_proj()
        with tc.spectator_scope("p_v"):
            v_proj()
        proj.close()

        # ---- per-head: attention (carrying next head's Q projection) ----
        att = ExitStack()
        vsp = att.enter_context(tc.tile_pool(name="vsp", bufs=2))
        qtp = att.enter_context(tc.tile_pool(name="qtp", bufs=2))
        exp_ = att.enter_context(tc.tile_pool(name="exp", bufs=7))
        accp = att.enter_context(tc.tile_pool(name="accp", bufs=4))
        dnp = att.enter_context(tc.tile_pool(name="dnp", bufs=2))
        qpp = att.enter_context(tc.tile_pool(name="qps", bufs=2, space="PSUM"))
        spp = att.enter_context(tc.tile_pool(name="sps", bufs=4, space="PSUM"))
        opp = att.enter_context(tc.tile_pool(name="ops", bufs=2, space="PSUM"))

        wo_pool = ExitStack()
        wo_sb = []

        def q_proj_emitter(h):
            """Returns (qt tile, step_fn). step_fn(n) emits up to n pending
            Q-projection matmuls for head h; emits the PSUM->SBUF copy when a
            512-col block completes. step_fn(None) flushes the current block."""
            qt = qtp.tile([128, t], BF16, tag="qt", name=f"qt{h}")
            state = {"blk": 0, "c": 0, "ps": None}

            def step(budget):
                n = 16 * n_tb - (state["blk"] * 16 + state["c"])
                if budget is not None:
                    n = min(budget, n)
                for _ in range(n):
                    blk, c = state["blk"], state["c"]
                    if c == 0:
                        state["ps"] = qpp.tile([128, 512], F32, tag="q",
                                               name=f"qp{h}_{blk}")
                    nc.tensor.matmul(
                        state["ps"], wq_t[c][:, h * 128:(h + 1) * 128],
                        xd_t[c][:, blk * 512:(blk + 1) * 512],
                        start=(c == 0), stop=(c == 15))
                    if c == 15:
                        dst = qt[:, blk * 512:(blk + 1) * 512]
                        if bq_b is not None:
                            nc.scalar.activation(dst, state["ps"], AF.Identity,
                                                 bias=bq_b[h])
                        elif blk % 2 == 0:
                            nc.scalar.activation(dst, state["ps"], AF.Identity)
                        else:
                            nc.vector.tensor_copy(dst, state["ps"])
                        state["blk"] += 1
                        state["c"] = 0
                    else:
                        state["c"] = c + 1
            return qt, step

        def oproj_emitter(oop):
            """Emits the output projection matmul-by-matmul so it can be
            woven into head 7's attention strips. Per tch: 4 psum tiles
            (2 from qps, 2 from sps), o-major accumulation, then copies +
            one out DMA."""
            state = {"tch": 0, "o": 0}

            def start_tch(tch):
                ps = []
                for cg in range(4):
                    pool = qpp if cg < 2 else spp
                    tag = "q" if cg < 2 else "s"
                    ps.append(pool.tile([128, 512], F32, tag=tag,
                                        name=f"op{tch}_{cg}"))
                state["ps"] = ps

            def step(budget):
                n = (n_tc - state["tch"]) * HPC - state["o"]
                if budget is not None:
                    n = min(budget, n)
                for _ in range(n):
                    tch, o = state["tch"], state["o"]
                    if o == 0:
                        start_tch(tch)
                    ps = state["ps"]
                    for cg in range(4):
                        nc.tensor.matmul(
                            ps[cg],
                            ot_h[o][:, tch * 128:(tch + 1) * 128],
                            wo_sb[o][:, cg * 512:(cg + 1) * 512],
                            start=(o == 0), stop=(o == HPC - 1))
                    if o == HPC - 1:
                        osb = oop.tile([128, DIM], F32, tag="os",
                                       name=f"oo{tch}")
                        for cg in range(4):
                            seg = osb[:, cg * 512:(cg + 1) * 512]
                            if cg % 2 == 0:
                                nc.scalar.activation(seg, ps[cg], AF.Identity)
                            else:
                                nc.vector.tensor_copy(seg, ps[cg])
                            if cg == 1:
                                nc.sync.dma_start(
                                    out=out[tch * 128:(tch + 1) * 128, 0:1024],
                                    in_=osb[:, 0:1024])
                        nc.sync.dma_start(
                            out=out[tch * 128:(tch + 1) * 128, 1024:2048],
                            in_=osb[:, 1024:2048])
                        state["tch"] += 1
                        state["o"] = 0
                    else:
                        state["o"] = o + 1
            return step

        with tc.spectator_scope("att"):
            qt0, step0 = q_proj_emitter(0)
            step0(None)  # head 0's Q runs standalone at attention start
            qts = {0: qt0}
            vts = {}
            oproj_step = None

            def issue_vt(h):
                vt = vsp.tile([128, n_tc * 128], BF16, tag="v", name=f"av{h}")
                nc.sync.dma_start(
                    out=vt.rearrange("p (i c) -> p i c", i=n_tc),
                    in_=v_s_r[:, :, h * 128:(h + 1) * 128])
                vts[h] = vt

            issue_vt(0)

            # global strip stream across heads and q-groups
            strip_desc = []
            per_head = 0
            for h in range(HPC):
                for g in range(n_g):
                    ni = 4 * g + 4
                    for i in range(ni):
                        o = 128 * (i - 4 * g) if i >= 4 * g else 0
                        strip_desc.append((h, g, i, o, ni))
            per_head = len(strip_desc) // HPC
            n_all = len(strip_desc)

            # per-head weave plans are decided lazily at head start
            weave_fn = [None]
            weave_plan = [None]
            gst = {}            # (h,g) -> fold state
            exs = [None] * n_all
            state = {"emit": None}

            def head_start(h):
                # flush previous head's carried Q projection
                if state["emit"] is not None:
                    state["emit"](None)
                    state["emit"] = None
                plan = [0] * per_head
                if h + 1 < HPC:
                    qts[h + 1], step = q_proj_emitter(h + 1)
                    for j in range(16 * n_tb):
                        plan[j % per_head] += 1
                    state["emit"] = step
                    weave_fn[0] = step
                    weave_plan[0] = plan
                    return
                # head 7: free Q residents, load wo, weave output projection
                qside.close()
                wop = wo_pool.enter_context(
                    tc.tile_pool(name="wop", bufs=1, side="right"))
                for o in range(HPC):
                    wt = wop.tile([128, DIM], BF16, tag=f"wo{o}",
                                  name=f"owo{o}")
                    nc.sync.dma_start(out=wt,
                                      in_=wo[o * 128:(o + 1) * 128, :])
                    wo_sb.append(wt)
                oop = wo_pool.enter_context(
                    tc.tile_pool(name="oo", bufs=2, side="right"))
                nonlocal_oproj = oproj_emitter(oop)
                state["oproj"] = nonlocal_oproj
                for j in range(4, per_head):
                    plan[j] = 2
                weave_fn[0] = nonlocal_oproj
                weave_plan[0] = plan

            def finish_g(h, g, st):
                acc = accp.tile([128, 512], F32, tag="a", name=f"acd{h}_{g}")
                nc.vector.tensor_copy(acc, st["diag"][0][0])
                for lv, tile_ in st["stack"]:
                    nc.vector.tensor_add(acc, acc, tile_)
                for ex, o in st["diag"][1:]:
                    nc.vector.tensor_add(acc[:, o:512], acc[:, o:512],
                                         ex[:, o:512])
                den = dnp.tile([128, 512], F32, tag="dn", name=f"dd{h}_{g}")
                nc.gpsimd.partition_all_reduce(den, acc, channels=128,
                                               reduce_op=RADD)
                dinv = dnp.tile([1, 512], F32, tag="di", name=f"di{h}_{g}")
                nc.vector.reciprocal_approx_fast(dinv, den[0:1, :])
                dnb = dnp.tile([128, 512], F32, tag="db", name=f"db{h}_{g}")
                nc.gpsimd.partition_broadcast(dnb, dinv)
                nc.vector.tensor_mul(ot_h[h][:, g * 512:(g + 1) * 512],
                                     st["pso"], dnb)

            def pv_stage(k):
                h, g, i, o, ni = strip_desc[k]
                if i == 0:
                    gst[(h, g)] = {
                        "pso": opp.tile([128, 512], F32, tag="o",
                                        name=f"apo{h}_{g}"),
                        "stack": [], "leaf": None, "diag": [], "nf": 0}
                st = gst[(h, g)]
                ex = exs[k]
                nc.tensor.matmul(
                    st["pso"][:, o:512],
                    vts[h][:, i * 128:(i + 1) * 128],
                    ex[:, o:512],
                    start=(i == 0), stop=(i == ni - 1))
                if i >= 4 * g:
                    st["diag"].append((ex, o))
                elif st["leaf"] is None:
                    st["leaf"] = ex
                else:
                    dst = accp.tile([128, 512], F32, tag="a",
                                    name=f"ac{h}_{g}_l{st['nf']}")
                    eng = nc.gpsimd if st["nf"] % 2 == 1 else nc.vector
                    eng.tensor_add(dst, st["leaf"], ex)
                    st["leaf"] = None
                    cur = (1, dst)
                    while st["stack"] and st["stack"][-1][0] == cur[0]:
                        lv, other = st["stack"].pop()
                        st["nf"] += 1
                        dst2 = accp.tile([128, 512], F32, tag="a",
                                         name=f"ac{h}_{g}_m{st['nf']}")
                        eng = nc.gpsimd if st["nf"] % 2 == 1 else nc.vector
                        eng.tensor_add(dst2, other, cur[1])
                        cur = (lv + 1, dst2)
                    st["stack"].append(cur)
                    st["nf"] += 1
                if i == ni - 1:
                    finish_g(h, g, st)
                    del gst[(h, g)]

            for k in range(n_all + 2):
                if k < n_all:
                    h, g, i, o, ni = strip_desc[k]
                    if g == 0 and i == 0:
                        head_start(h)
                    if g == 2 and i == 0 and h + 1 < HPC:
                        issue_vt(h + 1)
                    qt = qts[h]
                    pss = spp.tile([128, 512], F32, tag="s",
                                   name=f"aps{h}_{g}_{i}")
                    nc.tensor.matmul(
                        pss[:, o:512],
                        kt_h[h][:, i * 128:(i + 1) * 128],
                        qt[:, g * 512 + o:(g + 1) * 512],
                        start=True, stop=True)
                    ex = exp_.tile([128, 512], BF16, tag="e",
                                   name=f"ae{h}_{g}_{i}")
                    nc.scalar.activation(ex[:, o:512], pss[:, o:512],
                                         AF.Exp, scale=float(SCALE))
                    if i >= 4 * g:
                        nc.vector.tensor_mul(ex[:, o:o + 128],
                                             ex[:, o:o + 128], tri_sb)
                    exs[k] = ex
                    if weave_fn[0] is not None:
                        weave_fn[0](weave_plan[0][k % per_head])
                if k >= 2:
                    pv_stage(k - 2)

            # drain the rest of the output projection
            with tc.spectator_scope("oproj"):
                state["oproj"](None)
        att.close()
        wo_pool.close()

    nc.compile()
    return nc


def _host_tri():
    c = np.arange(128)[None, :]
    r = np.arange(128)[:, None]
    return (c >= r).astype(np.float32)


def _make_in_maps(inputs):
    import ml_dtypes

    bf16 = ml_dtypes.bfloat16
    encoder_x = np.asarray(inputs["encoder_x"], dtype=np.float32)
    decoder_x = np.asarray(inputs["decoder_x"], dtype=np.float32)
    W_kv = np.asarray(inputs["W_kv"], dtype=np.float32)
    b_kv = np.asarray(inputs["b_kv"], dtype=np.float32)
    W_q = np.asarray(inputs["W_q"], dtype=np.float32)
    b_q = np.asarray(inputs["b_q"], dtype=np.float32)
    W_o = np.asarray(inputs["W_o"], dtype=np.float32)

    add_bias_kq = bool(np.any(b_kv) or np.any(b_q))
    tri = _host_tri().astype(bf16)

    in_maps = []
    for core in range(N_CORES):
        b, hg = core // 2, core % 2
        s = hg * KC
        im = {
            "xeT": np.ascontiguousarray(encoder_x[b].T).astype(bf16),
            "xdT": np.ascontiguousarray(decoder_x[b].T).astype(bf16),
            "wk": np.ascontiguousarray(W_kv[:, s:s + KC]).astype(bf16),
            "wv": np.ascontiguousarray(W_kv[:, DIM + s:DIM + s + KC]).astype(bf16),
            "wq": np.ascontiguousarray(W_q[:, s:s + KC]).astype(bf16),
            "wo": np.ascontiguousarray(W_o[s:s + KC, :]).astype(bf16),
            "tri": tri,
        }
        if add_bias_kq:
            im["bk"] = np.ascontiguousarray(b_kv[s:s + KC][:, None])
            im["bq"] = np.ascontiguousarray(b_q[s:s + KC][:, None])
            im["bvb"] = np.ascontiguousarray(
                np.broadcast_to(b_kv[DIM + s:DIM + s + KC], (128, KC)).copy())
        in_maps.append(im)
    return in_maps


def kernel(encoder_x, decoder_x, W_kv, b_kv, W_q, b_q, W_o, b_o):
    from concourse.bass_utils import run_bass_kernel_spmd

    b_kv = np.asarray(b_kv, dtype=np.float32)
    b_q = np.asarray(b_q, dtype=np.float32)
    b_o = np.asarray(b_o, dtype=np.float32)

    add_bias_kq = bool(np.any(b_kv) or np.any(b_q))
    nc = _build(T, add_bias_kq=add_bias_kq)

    in_maps = _make_in_maps(dict(
        encoder_x=encoder_x, decoder_x=decoder_x, W_kv=W_kv, b_kv=b_kv,
        W_q=W_q, b_q=b_q, W_o=W_o, b_o=b_o))

    res = run_bass_kernel_spmd(nc, in_maps, core_ids=list(range(N_CORES)),
                               trace=False)
    out = np.empty((B, T, DIM), np.float32)
    for b in range(B):
        out[b] = (res.results[2 * b]["out"].astype(np.float64)
                  + res.results[2 * b + 1]["out"].astype(np.float64)
                  + b_o.astype(np.float64)).astype(np.float32)
    return out



# revision 37
# speedup vs baseline: 1.0715x; 1.0715x over previous
"""Cross-multi-head-attention (causal) Trainium2 Bass kernel, v3.

Problem: B=4, T=2048, C=2048, 16 heads x head_dim 128.
  kv = enc_x @ W_kv + b_kv ; q = dec_x @ W_q + b_q
  out = softmax_causal(q k^T / sqrt(hd)) v  -> concat heads -> @ W_o + b_o

Sharding over 8 cores: core c -> (batch b = c//2, head-group hg = c%2 of 8
heads). Host sums the two partials per batch and adds b_o.

v3 changes vs v2 (all aimed at PE idle/p-state, measured via TimelineSim):
- Warmup matmuls on a zeroed tile bridge the initial DMA wait so the PE
  p-state is fully ramped when real work starts; first k-proj loads are
  reordered (x chunk j0 split out, wk0 early).
- Head-0's Q projection block 0 is emitted inside the projection scope on
  the proj PSUM ring, so the PE rolls from v_proj straight into Q work
  while the proj psum copies drain (kills the 3.6us transition stall).
- The last v-block's staging tiles reuse dead x-buffer slots (tag "x"),
  removing a WAR on the serialized v_s write DMAs.
- Attention: score->PV pipeline distance 2 -> 4; exp always full width
  with a combined [zeros|tri] mask tile (one DVE mul zeroes stale psum
  region and applies the causal boundary); softmax-denominator is an
  eager bf16 pairwise-fold tree on DVE (2x mode) with a single f32 final
  merge, removing the long per-group finish chain that stalled diagonal
  PV matmuls.
- Output projection: per-512-col out DMAs for the last two row chunks to
  shorten the end drain.
"""
import sys

sys.path.insert(0, "/opt/trn_rl_repo")

import numpy as np

DIM = 2048
N_HEAD = 16
HEAD = DIM // N_HEAD  # 128
B = 4
T = 2048
HPC = 8               # heads per core
KC = HPC * HEAD       # 1024 projected cols per core
SCALE = 1.0 / np.sqrt(float(HEAD))
N_CORES = 8
DIST = 4              # score -> PV pipeline distance (strips)
N_WARM = 100          # warmup matmuls at kernel start


def _build(t=T, add_bias_kq=False):
    from contextlib import ExitStack

    import concourse.mybir as mybir
    from concourse import bacc
    from concourse import bass_isa
    from concourse.tile import TileContext

    F32 = mybir.dt.float32
    BF16 = mybir.dt.bfloat16
    AF = mybir.ActivationFunctionType
    RADD = bass_isa.ReduceOp.add

    n_tb = t // 512      # 512-col T blocks
    n_tc = t // 128      # 128-row T chunks
    n_g = t // 512       # q groups in attention

    nc = bacc.Bacc("TRN2", target_bir_lowering=False, debug=False, num_devices=1)
    xeT = nc.dram_tensor("xeT", [DIM, t], BF16, kind="ExternalInput").ap()
    xdT = nc.dram_tensor("xdT", [DIM, t], BF16, kind="ExternalInput").ap()
    wk = nc.dram_tensor("wk", [DIM, KC], BF16, kind="ExternalInput").ap()
    wv = nc.dram_tensor("wv", [DIM, KC], BF16, kind="ExternalInput").ap()
    wq = nc.dram_tensor("wq", [DIM, KC], BF16, kind="ExternalInput").ap()
    wo = nc.dram_tensor("wo", [KC, DIM], BF16, kind="ExternalInput").ap()
    # [zeros(384) | lower-tri(128)] combined stale-psum + causal mask
    mz = nc.dram_tensor("mz", [128, 512], BF16, kind="ExternalInput").ap()
    if add_bias_kq:
        bk = nc.dram_tensor("bk", [KC, 1], F32, kind="ExternalInput").ap()
        bq = nc.dram_tensor("bq", [KC, 1], F32, kind="ExternalInput").ap()
        bvb = nc.dram_tensor("bvb", [128, KC], F32, kind="ExternalInput").ap()
    out = nc.dram_tensor("out", [t, DIM], BF16, kind="ExternalOutput").ap()

    v_s = nc.dram_tensor("v_s", [t, KC], BF16, kind="Internal").ap()
    # [128p, 16i, 128c] view: element (p,i,c) = v_s[i*128+p, c]
    v_s_r = v_s.rearrange("(i p) c -> p i c", p=128)
    # [128p, 4j, t] views of x chunks: element (p,j,u) = x[(s*4+j)*128+p, u]
    xeT_r = xeT.rearrange("(s j p) u -> s p j u", j=4, p=128)

    with TileContext(nc) as tc, ExitStack() as top:
        glob = top.enter_context(tc.tile_pool(name="glob", bufs=1))
        warm = glob.tile([128, 128], BF16, tag="wrm", name="warm")
        q0a = glob.tile([128, 512], BF16, tag="q0a", name="q0a")
        q0b = glob.tile([128, 512], BF16, tag="q0b", name="q0b")
        bk_b = bq_b = bvb_sb = None
        if add_bias_kq:
            bk_sb = glob.tile([128, HPC], F32, tag="bk", name="bk_sb")
            bq_sb = glob.tile([128, HPC], F32, tag="bq", name="bq_sb")
            bvb_sb = glob.tile([128, KC], F32, tag="bvb", name="bvb_sb")
            for h in range(HPC):
                nc.sync.dma_start(out=bk_sb[:, h:h + 1],
                                  in_=bk[h * 128:(h + 1) * 128, :])
                nc.sync.dma_start(out=bq_sb[:, h:h + 1],
                                  in_=bq[h * 128:(h + 1) * 128, :])
            nc.sync.dma_start(out=bvb_sb, in_=bvb)
            bk_b = [bk_sb[:, h:h + 1] for h in range(HPC)]
            bq_b = [bq_sb[:, h:h + 1] for h in range(HPC)]

        # ---- persistent SBUF residents ----
        ktp = top.enter_context(tc.tile_pool(name="ktp", bufs=1))
        otp = top.enter_context(tc.tile_pool(name="otp", bufs=1))
        kt_h = [ktp.tile([128, t], BF16, tag=f"kt{h}", name=f"kt{h}")
                for h in range(HPC)]
        ot_h = [otp.tile([128, t], BF16, tag=f"ot{h}", name=f"ot{h}")
                for h in range(HPC)]

        # Q-side residents (xdT chunks + wq), freed before wo loads
        qside = ExitStack()
        xdp = qside.enter_context(tc.tile_pool(name="xdp", bufs=1, side="right"))
        wqp = qside.enter_context(tc.tile_pool(name="wqp", bufs=1, side="right"))
        xd_t = [xdp.tile([128, t], BF16, tag=f"xd{c}", name=f"xd{c}")
                for c in range(16)]
        wq_t = [wqp.tile([128, KC], BF16, tag=f"wq{c}", name=f"wq{c}")
                for c in range(16)]

        # ---- K/V projection (streamed xeT, bf16 weights) ----
        proj = ExitStack()
        xp = proj.enter_context(tc.tile_pool(name="px", bufs=2, side="right"))
        wp = proj.enter_context(tc.tile_pool(name="pw", bufs=16, side="right"))
        vo = proj.enter_context(tc.tile_pool(name="pvo", bufs=2, side="right"))
        pp = proj.enter_context(tc.tile_pool(name="pps", bufs=8, space="PSUM"))

        # warmup: keep the PE busy (and its p-state ramping) while the
        # first x / weight DMAs land
        nc.gpsimd.memset(warm, 0.0)
        wps = pp.tile([128, 512], F32, tag="p", name="warm_ps")
        for _ in range(N_WARM):
            nc.tensor.matmul(wps[:, 0:48], warm, warm[:, 0:48],
                             start=True, stop=True)

        def x_super(tb, s, pfx):
            """One DMA bringing xeT chunks 4s..4s+3, cols [tb*512,(tb+1)*512)
            into a [128, 4, 512] tile viewed as [128, 2048]."""
            x1 = xp.tile([128, 4 * 512], BF16, tag="x", name=f"{pfx}x{tb}_{s}")
            nc.sync.dma_start(
                out=x1.rearrange("p (j u) -> p j u", j=4),
                in_=xeT_r[s][:, :, tb * 512:(tb + 1) * 512])
            return x1

        def k_proj():
            wts = [None] * 16
            xts = [None] * 4
            # first loads ordered for earliest possible first matmul:
            # x(s=0,j=0) piece, wk0, x(s=0,j=1..3), wk1-3, then s=1..3
            x1 = xp.tile([128, 4 * 512], BF16, tag="x", name="kx0_0")
            xv = x1.rearrange("p (j u) -> p j u", j=4)
            nc.sync.dma_start(out=xv[:, 0:1, :], in_=xeT_r[0][:, 0:1, 0:512])
            wt = wp.tile([128, KC], BF16, tag="w", name="kw_0")
            nc.sync.dma_start(out=wt, in_=wk[0:128, :])
            wts[0] = wt
            nc.sync.dma_start(out=xv[:, 1:4, :], in_=xeT_r[0][:, 1:4, 0:512])
            for c in (1, 2, 3):
                wt = wp.tile([128, KC], BF16, tag="w", name=f"kw_{c}")
                nc.sync.dma_start(out=wt, in_=wk[c * 128:(c + 1) * 128, :])
                wts[c] = wt
            xts[0] = x1
            for s in (1, 2, 3):
                xts[s] = x_super(0, s, "k")
                for c in range(4 * s, 4 * s + 4):
                    wt = wp.tile([128, KC], BF16, tag="w", name=f"kw_{c}")
                    nc.sync.dma_start(out=wt, in_=wk[c * 128:(c + 1) * 128, :])
                    wts[c] = wt
            for tb in range(n_tb):
                if tb > 0:
                    xts = [x_super(tb, s, "k") for s in range(4)]
                ps = [pp.tile([128, 512], F32, tag="p", name=f"kp{tb}_{h}")
                      for h in range(HPC)]
                for ci in range(16):
                    xr = xts[ci // 4][:, (ci % 4) * 512:(ci % 4 + 1) * 512]
                    for h in range(HPC):
                        nc.tensor.matmul(
                            ps[h], wts[ci][:, h * 128:(h + 1) * 128], xr,
                            start=(ci == 0), stop=(ci == 15))
                for h in range(HPC):
                    dst = kt_h[h][:, tb * 512:(tb + 1) * 512]
                    if bk_b is not None:
                        nc.scalar.activation(dst, ps[h], AF.Identity,
                                             bias=bk_b[h])
                    elif h % 2 == 0:
                        nc.scalar.activation(dst, ps[h], AF.Identity)
                    else:
                        nc.vector.tensor_copy(dst, ps[h])

        def v_proj():
            xts0 = [x_super(0, s, "v") for s in range(4)]
            wts = []
            for c in range(16):
                wt = wp.tile([128, KC], BF16, tag="w", name=f"vw_{c}")
                nc.sync.dma_start(out=wt, in_=wv[c * 128:(c + 1) * 128, :])
                wts.append(wt)
            # prefetch Q-side residents behind the wv loads
            for c in range(16):
                nc.sync.dma_start(out=wq_t[c],
                                  in_=wq[c * 128:(c + 1) * 128, :])
                nc.sync.dma_start(out=xd_t[c],
                                  in_=xdT[c * 128:(c + 1) * 128, :])
            xts = xts0
            for tb in range(n_tb):
                # two 4-psum halves per tb: half A's copies drain during
                # half B's matmuls, so the next tb's psum WAR never stalls
                for half in range(2):
                    tss = (0, 1) if half == 0 else (2, 3)
                    ps = {(ts, vg): pp.tile([128, 512], F32, tag="p",
                                            name=f"vp{tb}_{ts}_{vg}")
                          for ts in tss for vg in range(2)}
                    for c in range(16):
                        xr = xts[c // 4][:, (c % 4) * 512:(c % 4 + 1) * 512]
                        for ts in tss:
                            for vg in range(2):
                                nc.tensor.matmul(
                                    ps[(ts, vg)],
                                    xr[:, ts * 128:(ts + 1) * 128],
                                    wts[c][:, vg * 512:(vg + 1) * 512],
                                    start=(c == 0), stop=(c == 15))
                    # issue next tb's x supers BEFORE the copies/v_s writes
                    # so their HWDGE generation isn't queued behind v_s
                    # writes that wait on this tb's staging copies
                    if half == 1 and tb + 1 < n_tb:
                        xts = [x_super(tb + 1, s, "v") for s in range(4)]
                    for ts in tss:
                        # last tb, last two chunks: reuse dead x slots so
                        # the staging tile never WARs a pending v_s write
                        if tb == n_tb - 1 and ts >= 2:
                            st = xp.tile([128, KC], BF16, tag="x",
                                         name=f"vo{tb}_{ts}")
                        else:
                            st = vo.tile([128, KC], BF16, tag="vo",
                                         name=f"vo{tb}_{ts}")
                        for vg in range(2):
                            seg = st[:, vg * 512:(vg + 1) * 512]
                            if bvb_sb is not None:
                                nc.vector.tensor_add(
                                    seg, ps[(ts, vg)],
                                    bvb_sb[:, vg * 512:(vg + 1) * 512])
                            elif (ts * 2 + vg) % 2 == 0:
                                nc.scalar.activation(seg, ps[(ts, vg)],
                                                     AF.Identity)
                            else:
                                nc.vector.tensor_copy(seg, ps[(ts, vg)])
                        # SWDGE queue: keeps SP.SEQ free for x supers and
                        # tile bookkeeping, and off the HWDGE ring
                        nc.gpsimd.dma_start(
                            out=v_s[tb * 512 + ts * 128:
                                    tb * 512 + (ts + 1) * 128, :],
                            in_=st)

        with tc.spectator_scope("p_k"):
            k_proj()
        with tc.spectator_scope("p_v"):
            v_proj()

        # Q head-0 blocks 0-1 on the proj psum ring: the PE rolls straight
        # from v_proj's last matmul into these (their psum slots WAR only
        # early drain copies), covering the proj->att psum pool boundary.
        # Block 1's matmuls also cover block 0's copy.
        qp0 = pp.tile([128, 512], F32, tag="p", name="q0ps")
        for c in range(16):
            nc.tensor.matmul(qp0, wq_t[c][:, 0:128], xd_t[c][:, 0:512],
                             start=(c == 0), stop=(c == 15))
        if bq_b is not None:
            nc.scalar.activation(q0a, qp0, AF.Identity, bias=bq_b[0])
        else:
            nc.scalar.activation(q0a, qp0, AF.Identity)
        qp1 = pp.tile([128, 512], F32, tag="p", name="q1ps")
        for c in range(16):
            nc.tensor.matmul(qp1, wq_t[c][:, 0:128], xd_t[c][:, 512:1024],
                             start=(c == 0), stop=(c == 15))
        if bq_b is not None:
            nc.scalar.activation(q0b, qp1, AF.Identity, bias=bq_b[0])
        else:
            nc.scalar.activation(q0b, qp1, AF.Identity)
        proj.close()

        # ---- per-head: attention (carrying next head's Q projection) ----
        att = ExitStack()
        vsp = att.enter_context(tc.tile_pool(name="vsp", bufs=2))
        qtp = att.enter_context(tc.tile_pool(name="qtp", bufs=2))
        exp_ = att.enter_context(tc.tile_pool(name="exp", bufs=8))
        accp = att.enter_context(tc.tile_pool(name="accp", bufs=5))
        dnp = att.enter_context(tc.tile_pool(name="dnp", bufs=2))
        qpp = att.enter_context(tc.tile_pool(name="qps", bufs=2, space="PSUM"))
        spp = att.enter_context(tc.tile_pool(name="sps", bufs=4, space="PSUM"))
        opp = att.enter_context(tc.tile_pool(name="ops", bufs=2, space="PSUM"))

        mz_sb = vsp.tile([128, 512], BF16, tag="mz", bufs=1, name="mz_sb")
        nc.gpsimd.dma_start(out=mz_sb, in_=mz)

        wo_pool = ExitStack()
        wo_sb = []

        def q_proj_emitter(h, blocks=None, pool=qpp, start_blk=0):
            """Returns (blocks, step_fn). step_fn(n) emits up to n pending
            Q-projection matmuls for head h; emits the PSUM->SBUF copy when a
            512-col block completes. step_fn(None) flushes."""
            if blocks is None:
                qt = qtp.tile([128, t], BF16, tag="qt", name=f"qt{h}")
                blocks = [qt[:, g * 512:(g + 1) * 512] for g in range(n_g)]
            state = {"blk": start_blk, "c": 0, "ps": None}
            total = 16 * (n_tb - start_blk)

            def step(budget):
                n = 16 * n_tb - (state["blk"] * 16 + state["c"])
                if budget is not None:
                    n = min(budget, n)
                for _ in range(n):
                    blk, c = state["blk"], state["c"]
                    if c == 0:
                        state["ps"] = pool.tile([128, 512], F32, tag="q",
                                                name=f"qp{h}_{blk}")
                    nc.tensor.matmul(
                        state["ps"], wq_t[c][:, h * 128:(h + 1) * 128],
                        xd_t[c][:, blk * 512:(blk + 1) * 512],
                        start=(c == 0), stop=(c == 15))
                    if c == 15:
                        dst = blocks[blk]
                        if bq_b is not None:
                            nc.scalar.activation(dst, state["ps"], AF.Identity,
                                                 bias=bq_b[h])
                        else:
                            nc.vector.tensor_copy(dst, state["ps"])
                        state["blk"] += 1
                        state["c"] = 0
                    else:
                        state["c"] = c + 1
            return blocks, step, total

        def oproj_emitter(oop):
            """Emits the output projection matmul-by-matmul so it can be
            woven into head 7's attention strips. Per tch: 4 psum tiles
            (2 from qps, 2 from sps), o-major accumulation, then copies +
            out DMAs (per-cg DMAs for the last two tchs)."""
            state = {"tch": 0, "o": 0}

            def start_tch(tch):
                ps = []
                for cg in range(4):
                    pool = qpp if cg < 2 else spp
                    tag = "q" if cg < 2 else "s"
                    ps.append(pool.tile([128, 512], F32, tag=tag,
                                        name=f"op{tch}_{cg}"))
                state["ps"] = ps

            def step(budget):
                n = (n_tc - state["tch"]) * HPC - state["o"]
                if budget is not None:
                    n = min(budget, n)
                for _ in range(n):
                    tch, o = state["tch"], state["o"]
                    if o == 0:
                        start_tch(tch)
                    ps = state["ps"]
                    for cg in range(4):
                        nc.tensor.matmul(
                            ps[cg],
                            ot_h[o][:, tch * 128:(tch + 1) * 128],
                            wo_sb[o][:, cg * 512:(cg + 1) * 512],
                            start=(o == 0), stop=(o == HPC - 1))
                    if o == HPC - 1:
                        osb = oop.tile([128, DIM], BF16, tag="os",
                                       name=f"oo{tch}")
                        if tch == n_tc - 1:
                            # final chunk: cg3 copied first and given its
                            # own queue so the very last write's chain
                            # (copy -> gen -> transfer -> sem) is shortest
                            cp_eng = {3: nc.scalar, 2: nc.vector,
                                      1: nc.scalar, 0: nc.vector}
                            dma_eng = {3: nc.sync, 2: nc.gpsimd,
                                       1: nc.scalar, 0: nc.scalar}
                            for cg in (3, 2, 1, 0):
                                seg = osb[:, cg * 512:(cg + 1) * 512]
                                if cp_eng[cg] is nc.scalar:
                                    nc.scalar.activation(seg, ps[cg],
                                                         AF.Identity)
                                else:
                                    nc.vector.tensor_copy(seg, ps[cg])
                                dma_eng[cg].dma_start(
                                    out=out[tch * 128:(tch + 1) * 128,
                                            cg * 512:(cg + 1) * 512],
                                    in_=seg)
                        else:
                            split = tch == n_tc - 2
                            for cg in range(4):
                                seg = osb[:, cg * 512:(cg + 1) * 512]
                                if cg % 2 == 0:
                                    nc.scalar.activation(seg, ps[cg],
                                                         AF.Identity)
                                else:
                                    nc.vector.tensor_copy(seg, ps[cg])
                                if split:
                                    eng = nc.sync if cg % 2 == 0 else nc.gpsimd
                                    eng.dma_start(
                                        out=out[tch * 128:(tch + 1) * 128,
                                                cg * 512:(cg + 1) * 512],
                                        in_=seg)
                                elif cg == 1:
                                    nc.sync.dma_start(
                                        out=out[tch * 128:(tch + 1) * 128,
                                                0:1024],
                                        in_=osb[:, 0:1024])
                            if not split:
                                nc.sync.dma_start(
                                    out=out[tch * 128:(tch + 1) * 128,
                                            1024:2048],
                                    in_=osb[:, 1024:2048])
                        state["tch"] += 1
                        state["o"] = 0
                    else:
                        state["o"] = o + 1
            return step

        with tc.spectator_scope("att"):
            q0_blocks = None
            qts = {}
            vts = {}

            def issue_vt(h):
                vt = vsp.tile([128, n_tc * 128], BF16, tag="v", name=f"av{h}")
                nc.sync.dma_start(
                    out=vt.rearrange("p (i c) -> p i c", i=n_tc),
                    in_=v_s_r[:, :, h * 128:(h + 1) * 128])
                vts[h] = vt

            # head 0: blocks 0-1 were computed in proj scope; blocks 2-3 now
            qt0_rest = qtp.tile([128, t], BF16, tag="qt", name="qt0")
            q0_blocks = [q0a, q0b] + [qt0_rest[:, g * 512:(g + 1) * 512]
                                      for g in range(2, n_g)]
            qts[0], step0, _ = q_proj_emitter(0, blocks=q0_blocks, start_blk=2)
            step0(None)
            issue_vt(0)

            # global strip stream across heads and q-groups
            strip_desc = []
            for h in range(HPC):
                for g in range(n_g):
                    ni = 4 * g + 4
                    for i in range(ni):
                        o = 128 * (i - 4 * g) if i >= 4 * g else 0
                        strip_desc.append((h, g, i, o, ni))
            per_head = len(strip_desc) // HPC
            n_all = len(strip_desc)

            weave_fn = [None]
            weave_plan = [None]
            gst = {}            # (h,g) -> fold state
            exs = [None] * n_all
            state = {"emit": None}

            def head_start(h):
                # flush previous head's carried Q projection
                if state["emit"] is not None:
                    state["emit"](None)
                    state["emit"] = None
                plan = [0] * per_head
                if h + 1 < HPC:
                    qts[h + 1], step, total = q_proj_emitter(h + 1)
                    # front-load: the head's first strips are diagonal
                    # (little PE work) while Act grinds exps — give the PE
                    # extra weave there. For head 7's Q, finish by strip 35
                    # so the Q residents can be freed early for wo loads.
                    for j in range(total):
                        if h + 1 == HPC - 1:
                            plan[j // 2 if j < 56 else 28 + (j - 56)] += 1
                        elif j < 16:
                            plan[j // 2] += 1
                        elif j < 48:
                            plan[8 + (j - 16) // 2] += 1
                        else:
                            plan[24 + (j - 48)] += 1
                    state["emit"] = step
                    weave_fn[0] = step
                    weave_plan[0] = plan
                    return
                # head 7: wo was loaded at head 6 strip 36 (early_close).
                # Each oproj "step" is 4 matmuls for one (tch, o); pace so
                # every ot7 read is EMITTED after its finish_g write:
                # o=7 of tch0 (step 8) after iter 7, g1 cols (step 33)
                # after iter 15, g2 cols (step 65) after iter 27, g3 cols
                # (steps 97+) drain-only.
                nonlocal_oproj = oproj_emitter(state["oop"])
                state["oproj"] = nonlocal_oproj
                for j in range(6, 22):
                    plan[j] = 2
                for j in range(22, 38):
                    plan[j] = 4
                weave_fn[0] = nonlocal_oproj
                weave_plan[0] = plan

            def early_close():
                # head 7's Q is done: free the Q residents and start the
                # wo loads so oproj can weave from head 7's first strips
                if state["emit"] is not None:
                    state["emit"](None)
                    state["emit"] = None
                qside.close()
                wop = wo_pool.enter_context(
                    tc.tile_pool(name="wop", bufs=1, side="right"))
                for o in range(HPC):
                    wt = wop.tile([128, DIM], BF16, tag=f"wo{o}",
                                  name=f"owo{o}")
                    nc.sync.dma_start(out=wt,
                                      in_=wo[o * 128:(o + 1) * 128, :])
                    wo_sb.append(wt)
                state["oop"] = wo_pool.enter_context(
                    tc.tile_pool(name="oo", bufs=3, side="right"))

            def finish_g(h, g, st):
                assert st["leaf"] is None
                tiles = [tile_ for _, tile_ in st["stack"]]
                st["stack"] = []
                # merge any stack remnants in bf16, final merge to f32
                while len(tiles) > 2:
                    dst = accp.tile([128, 512], BF16, tag="a",
                                    name=f"fm{h}_{g}_{len(tiles)}")
                    nc.vector.tensor_add(dst, tiles[-1], tiles[-2])
                    tiles = tiles[:-2] + [dst]
                root = accp.tile([128, 512], F32, tag="af", bufs=2,
                                 name=f"rt{h}_{g}")
                if len(tiles) == 2:
                    nc.vector.tensor_add(root, tiles[0], tiles[1])
                else:
                    nc.vector.tensor_copy(root, tiles[0])
                den = dnp.tile([128, 512], F32, tag="dn", name=f"dd{h}_{g}")
                nc.gpsimd.partition_all_reduce(den, root, channels=128,
                                               reduce_op=RADD)
                nc.vector.reciprocal_approx_fast(den[0:1, :], den[0:1, :])
                dnb = dnp.tile([128, 512], F32, tag="db", name=f"db{h}_{g}")
                nc.gpsimd.partition_broadcast(dnb, den[0:1, :])
                nc.vector.tensor_mul(ot_h[h][:, g * 512:(g + 1) * 512],
                                     st["pso"], dnb)

            def pv_stage(k):
                h, g, i, o, ni = strip_desc[k]
                if i == 0:
                    gst[(h, g)] = {
                        "pso": opp.tile([128, 512], F32, tag="o",
                                        name=f"apo{h}_{g}"),
                        "stack": [], "leaf": None, "nf": 0}
                st = gst[(h, g)]
                ex = exs[k]
                nc.tensor.matmul(
                    st["pso"][:, o:512],
                    vts[h][:, i * 128:(i + 1) * 128],
                    ex[:, o:512],
                    start=(i == 0), stop=(i == ni - 1))
                # eager bf16 pairwise fold of the full-width ex tiles
                if st["leaf"] is None:
                    st["leaf"] = ex
                else:
                    dst = accp.tile([128, 512], BF16, tag="a",
                                    name=f"ac{h}_{g}_l{st['nf']}")
                    nc.vector.tensor_add(dst, st["leaf"], ex)
                    st["leaf"] = None
                    cur = (1, dst)
                    while st["stack"] and st["stack"][-1][0] == cur[0]:
                        lv, other = st["stack"].pop()
                        st["nf"] += 1
                        dst2 = accp.tile([128, 512], BF16, tag="a",
                                         name=f"ac{h}_{g}_m{st['nf']}")
                        nc.vector.tensor_add(dst2, other, cur[1])
                        cur = (lv + 1, dst2)
                    st["stack"].append(cur)
                    st["nf"] += 1
                if i == ni - 1:
                    finish_g(h, g, st)
                    del gst[(h, g)]

            for k in range(n_all + DIST):
                if k < n_all:
                    h, g, i, o, ni = strip_desc[k]
                    if g == 0 and i == 0:
                        head_start(h)
                    # prefetch next head's V early so head 7's vt DMA
                    # doesn't compete with the wo weight loads on HWDGE
                    if g == 0 and i == 1 and h + 1 < HPC:
                        issue_vt(h + 1)
                    if h == HPC - 2 and g == 3 and i == 12:
                        early_close()
                    qb = qts[h][g]
                    pss = spp.tile([128, 512], F32, tag="s",
                                   name=f"aps{h}_{g}_{i}")
                    nc.tensor.matmul(
                        pss[:, o:512],
                        kt_h[h][:, i * 128:(i + 1) * 128],
                        qb[:, o:512],
                        start=True, stop=True)
                    ex = exp_.tile([128, 512], BF16, tag="e",
                                   name=f"ae{h}_{g}_{i}")
                    # narrow exp on the computed region; zero [0:o) on the
                    # (idle) Pool engine so the bf16 fold tree stays valid
                    nc.scalar.activation(ex[:, o:512], pss[:, o:512],
                                         AF.Exp, scale=float(SCALE))
                    if i >= 4 * g:
                        if o > 0:
                            nc.gpsimd.memset(ex[:, 0:o], 0.0)
                        nc.vector.tensor_mul(ex[:, o:o + 128],
                                             ex[:, o:o + 128],
                                             mz_sb[:, 384:512])
                    exs[k] = ex
                    if weave_fn[0] is not None:
                        weave_fn[0](weave_plan[0][k % per_head])
                if k >= DIST:
                    pv_stage(k - DIST)

            # drain the rest of the output projection
            with tc.spectator_scope("oproj"):
                state["oproj"](None)
        att.close()
        wo_pool.close()

    nc.compile()
    return nc


def _host_mask():
    """[zeros(128,384) | tri(128,128)]: col c of the tri block allows key
    row r iff c >= r (query >= key within the diagonal 128-block)."""
    c = np.arange(128)[None, :]
    r = np.arange(128)[:, None]
    tri = (c >= r).astype(np.float32)
    return np.concatenate([np.zeros((128, 384), np.float32), tri], axis=1)


def _make_in_maps(inputs):
    import ml_dtypes

    bf16 = ml_dtypes.bfloat16
    encoder_x = np.asarray(inputs["encoder_x"], dtype=np.float32)
    decoder_x = np.asarray(inputs["decoder_x"], dtype=np.float32)
    W_kv = np.asarray(inputs["W_kv"], dtype=np.float32)
    b_kv = np.asarray(inputs["b_kv"], dtype=np.float32)
    W_q = np.asarray(inputs["W_q"], dtype=np.float32)
    b_q = np.asarray(inputs["b_q"], dtype=np.float32)
    W_o = np.asarray(inputs["W_o"], dtype=np.float32)

    add_bias_kq = bool(np.any(b_kv) or np.any(b_q))
    mzv = _host_mask().astype(bf16)

    in_maps = []
    for core in range(N_CORES):
        b, hg = core // 2, core % 2
        s = hg * KC
        im = {
            "xeT": np.ascontiguousarray(encoder_x[b].T).astype(bf16),
            "xdT": np.ascontiguousarray(decoder_x[b].T).astype(bf16),
            "wk": np.ascontiguousarray(W_kv[:, s:s + KC]).astype(bf16),
            "wv": np.ascontiguousarray(W_kv[:, DIM + s:DIM + s + KC]).astype(bf16),
            "wq": np.ascontiguousarray(W_q[:, s:s + KC]).astype(bf16),
            "wo": np.ascontiguousarray(W_o[s:s + KC, :]).astype(bf16),
            "mz": mzv,
        }
        if add_bias_kq:
            im["bk"] = np.ascontiguousarray(b_kv[s:s + KC][:, None])
            im["bq"] = np.ascontiguousarray(b_q[s:s + KC][:, None])
            im["bvb"] = np.ascontiguousarray(
                np.broadcast_to(b_kv[DIM + s:DIM + s + KC], (128, KC)).copy())
        in_maps.append(im)
    return in_maps


def kernel(encoder_x, decoder_x, W_kv, b_kv, W_q, b_q, W_o, b_o):
    from concourse.bass_utils import run_bass_kernel_spmd

    b_kv = np.asarray(b_kv, dtype=np.float32)
    b_q = np.asarray(b_q, dtype=np.float32)
    b_o = np.asarray(b_o, dtype=np.float32)

    add_bias_kq = bool(np.any(b_kv) or np.any(b_q))
    nc = _build(T, add_bias_kq=add_bias_kq)

    in_maps = _make_in_maps(dict(
        encoder_x=encoder_x, decoder_x=decoder_x, W_kv=W_kv, b_kv=b_kv,
        W_q=W_q, b_q=b_q, W_o=W_o, b_o=b_o))

    res = run_bass_kernel_spmd(nc, in_maps, core_ids=list(range(N_CORES)),
                               trace=False)
    out = np.empty((B, T, DIM), np.float32)
    for b in range(B):
        out[b] = (res.results[2 * b]["out"].astype(np.float64)
                  + res.results[2 * b + 1]["out"].astype(np.float64)
                  + b_o.astype(np.float64)).astype(np.float32)
    return out
